# revision 75
# baseline (speedup 1.0000x reference)
"""Trainium2 Bass kernel for nn_Blocks_86096914416144.

Spiking-neuron block scan: T=1024 steps in 128 sequential blocks of tb=8,
B=32, N=1024, fp32. Sharding: channel dim N split 8 ways across cores
(pure data parallel; per-channel params private per core).

Per-core layout: SBUF tiles [128 partitions = channel n, free = (bh:2, bl:16,
tau:8)] with tau innermost so the within-block recurrences run as DVE
tensor_tensor_scan ops. x arrives [T,B,Nsh] (n contiguous) and is DMA'd as
[(bl,tau), (bh,n)] tiles, then PE-transposed into channel-major layout.

Algorithm per block k (r-formulation; validated vs reference in numpy):
  d2[slot 1..8] = (rd_prev >= 0) * x  [refractory mask; rd = r - r[last] from
                                       the previous block, precomputed on Pool]
  d2[slot0] = omf * m_prev[last]      [v_init; omf = relu(1-sumr) on ACT]
  m = scan(state = beta*state + d2)   [membrane; slot0 pattern=0 resets]
  f = (m - 1) > bpa                   [spikes pre-refractory; bpa=bb*p^(t+1)*a]
  r = scan(state = max(seg*state, f)) [cummax: any spike at or before tau]
  spikes = D @ transpose(r)           [first-difference matmul on PE = the
                                       first spike only, in output layout]
  sumr = sum_tau r                    [= 8 - tau* if spiked else 0]
  u = exp(ln(p)*(sumr-2)) * r[last]   [= p^(6-tau*) if spiked else 0]
  a' = a*p^8 + u                      [adaptation carry, closed form]

Engine split: DVE runs only the state-critical chain (d2-STT, m-scan, f-STT,
r-scan, sumr-reduce); Pool runs bpa/rd/gate/a-update/v_init; ACT runs the
exp/omf/amid, PSUM->SBUF copies and output DMA issue; PE runs the input
transposes and the spike first-difference matmul (bf16 D matrix).
"""

import os
import sys

import numpy as np

T_LEN = 1024
TB = 8
TBP = TB + 1  # 9 slots: slot 0 carries v_init into the scan
B = 32
N = 1024
NCORES = 8
NSH = N // NCORES  # 128 channels per core
BH = 2
BL = 16  # b = bh*16 + bl
FD = B * TB  # 256
FD9 = B * TBP  # 288

_MODULE_CACHE = {}


def _import_concourse():
    try:
        import concourse  # noqa: F401
    except ImportError:
        for p in ("/opt/trn_rl_repo", "/root/.axon_site/_ro/trn_rl_repo"):
            if os.path.isdir(p) and p not in sys.path:
                sys.path.insert(0, p)
        import concourse  # noqa: F401


def _build_module(t_len, repeats=1, rep_reset=True, variant="full", cyc=1):
    """Build + compile the per-core Bass module (SPMD: same NEFF, 8 cores).

    repeats>1 re-runs the whole scan (state re-initialised) for differential
    wall-clock timing; the final repeat's output is the correct result.
    cyc>1 unrolls the block loop cyc times over the same input blocks
    (k % nblk) — fully unrolled steady-state for timing, no loop barrier."""
    _import_concourse()
    from contextlib import ExitStack, nullcontext

    import concourse.bacc as bacc
    import concourse.bass as bass
    import concourse.tile as tile
    from concourse import masks, mybir

    nblk = t_len // TB
    f32 = mybir.dt.float32
    Alu = mybir.AluOpType
    Act = mybir.ActivationFunctionType

    nc = bacc.Bacc("TRN2", target_bir_lowering=False, debug=False)

    x_d = nc.dram_tensor("x", [t_len, B, NSH], f32, kind="ExternalInput")
    bpat_d = nc.dram_tensor("bpat", [128, FD9], f32, kind="ExternalInput")
    segpat_d = nc.dram_tensor("segpat", [128, FD], f32, kind="ExternalInput")
    ppowt_d = nc.dram_tensor("ppowt", [128, FD], f32, kind="ExternalInput")
    dmat_d = nc.dram_tensor(
        "dmat", [128, 128], mybir.dt.bfloat16, kind="ExternalInput"
    )
    # prm columns: 0=p^8, 1=ln(p), 2=-2*ln(p), 3=spare
    prm_d = nc.dram_tensor("prm", [128, 4], f32, kind="ExternalInput")
    y_d = nc.dram_tensor("y", [t_len, B, NSH], f32, kind="ExternalOutput")

    def r4(ap):  # [128, 256] -> [128, 2, 16, 8]
        return ap.rearrange("p (bh bl t) -> p bh bl t", bh=BH, t=TB)

    def r3(ap, t=TB):  # [128, F] -> [128, 32, t]
        return ap.rearrange("p (b t) -> p b t", t=t)

    def dram_block_ap(tens_ap, k):
        # [(bl, tau) partition-order, (bh, n) free-order] view of block k of a
        # [t_len, B, NSH] dram tensor; element order matches a [128, 256] tile.
        return bass.AP(
            tensor=tens_ap.tensor,
            offset=k * TB * B * NSH,
            ap=[[NSH, BL], [B * NSH, TB], [BL * NSH, BH], [1, NSH]],
        )

    with tile.TileContext(nc) as tc, ExitStack() as ctx:
        const = ctx.enter_context(tc.tile_pool(name="const", bufs=1))
        state = ctx.enter_context(tc.tile_pool(name="state", bufs=1))
        xp = ctx.enter_context(tc.tile_pool(name="xp", bufs=6))
        outp = ctx.enter_context(tc.tile_pool(name="outp", bufs=6))
        mp = ctx.enter_context(tc.tile_pool(name="mp", bufs=3))
        rp = ctx.enter_context(tc.tile_pool(name="rp", bufs=3))
        wk = ctx.enter_context(tc.tile_pool(name="wk", bufs=3))
        psin = ctx.enter_context(tc.tile_pool(name="psin", bufs=2, space="PSUM"))
        psout = ctx.enter_context(tc.tile_pool(name="psout", bufs=2, space="PSUM"))

        # constants
        bpat = const.tile([128, FD9], f32)
        segpat = const.tile([128, FD], f32)
        ppowt = const.tile([128, FD], f32)
        prm = const.tile([128, 4], f32)
        ident = const.tile([128, 128], f32)
        dmat = const.tile([128, 128], mybir.dt.bfloat16)
        nc.sync.dma_start(out=bpat[:], in_=bpat_d[:])
        nc.sync.dma_start(out=segpat[:], in_=segpat_d[:])
        nc.sync.dma_start(out=ppowt[:], in_=ppowt_d[:])
        nc.sync.dma_start(out=prm[:], in_=prm_d[:])
        nc.sync.dma_start(out=dmat[:], in_=dmat_d[:])
        masks.make_identity(nc, ident[:])
        p8_col = prm[:, 0:1]
        lnp_col = prm[:, 1:2]
        m2lnp_col = prm[:, 2:3]
        one_b = prm[:, 3:4].broadcast_to([128, B])
        p8_b = p8_col.broadcast_to([128, B])

        # persistent per-(n,b) state
        a0 = state.tile([128, B], f32)
        a1 = state.tile([128, B], f32)
        nc.vector.memset(a0[:], 0.0)

        r_prev = None

        for rep in range(1 if repeats > 1 else 1):
          loop_cm = tc.For_i(0, repeats, 1) if repeats > 1 else nullcontext()
          with loop_cm:
            d2 = wk.tile([128, FD9], f32, tag="d2")
            nc.vector.memset(r3(d2[:], TBP)[:, :, 0:1], 0.0)
            rd_prev = wk.tile([128, FD], f32, tag="rd")
            nc.vector.memset(rd_prev[:], 0.0)
            if repeats > 1 and rep_reset:
                nc.vector.memset(a0[:], 0.0)
            for kk in range(nblk * cyc):
               k = kk % nblk
               a_in = (a0, a1)[kk % 2]
               a_out = (a0, a1)[(kk + 1) % 2]
               do_dma = variant in ("full", "dma")
               do_cmp = variant in ("full", "dve")
               full = variant == "full"

               # ---- threshold term bpa = (bb*p^(t+1))*a  (Pool TT) ----
               if full:
                   bpa = wk.tile([128, FD], f32, tag="bpa")
                   a_b = a_in[:].rearrange("p (bh bl) -> p bh bl", bh=BH)
                   a_b = a_b.unsqueeze(3).broadcast_to([128, BH, BL, TB])
                   nc.gpsimd.tensor_tensor(
                       out=r4(bpa[:]), in0=r4(ppowt[:]), in1=a_b, op=Alu.mult,
                   )
                   # a*p8 term: depends only on a_in, run early on ACT
                   amid = wk.tile([128, B], f32, tag="amid")
                   nc.scalar.mul(amid[:], a_in[:], p8_col)
               else:
                   bpa = segpat

               # ---- input: DMA block + PE transpose into 9-slot layout ----
               # xT9 free layout (bh, bl, t9): slots 1..8 = x block (PE
               # transpose, strided out), slot 0 = m_last of prev block
               # (ACT copy) so one STT builds the whole scan input.
               if do_dma:
                   xN = xp.tile([128, FD], f32)
                   nc.sync.dma_start(out=xN[:], in_=dram_block_ap(x_d[:], k))
               if full:
                   xT = psin.tile([128, FD], f32)
                   nc.tensor.transpose(xT[:, 0:128], xN[:, 0:128], ident[:])
                   nc.tensor.transpose(xT[:, 128:256], xN[:, 128:256], ident[:])
                   xsb = xT
               else:
                   xsb = segpat

               if do_cmp:
                   # ---- d2 slots 1..8 = (rd_prev >= 0) * x (one DVE STT) ----
                   d2s = d2[:].rearrange(
                       "p (bh bl t) -> p bh bl t", bh=BH, t=TBP
                   )[:, :, :, 1:TBP]
                   nc.vector.scalar_tensor_tensor(
                       out=d2s, in0=r4(rd_prev[:]), scalar=0.0,
                       in1=r4(xsb[:]), op0=Alu.is_ge, op1=Alu.mult,
                   )

                   # ---- membrane scan (DVE) ----
                   m = mp.tile([128, FD9], f32)
                   nc.vector.tensor_tensor_scan(
                       out=m[:], data0=bpat[:], data1=d2[:], initial=0.0,
                       op0=Alu.mult, op1=Alu.add,
                   )

                   # ---- faulty f = (m-1) > bpa (DVE STT) ----
                   f = wk.tile([128, FD], f32, tag="f")
                   ms = m[:].rearrange(
                       "p (bh bl t) -> p bh bl t", bh=BH, t=TBP
                   )[:, :, :, 1:TBP]
                   nc.vector.scalar_tensor_tensor(
                       out=r4(f[:]), in0=ms, scalar=1.0, in1=r4(bpa[:]),
                       op0=Alu.subtract, op1=Alu.is_gt,
                   )

                   # ---- r = segmented cummax of f (DVE scan) ----
                   r = rp.tile([128, FD], f32)
                   nc.vector.tensor_tensor_scan(
                       out=r[:], data0=segpat[:], data1=f[:], initial=0.0,
                       op0=Alu.mult, op1=Alu.max,
                   )
                   r_end = r3(r[:])[:, :, TB - 1:TB].squeeze(2)  # [128, 32]

                   # ---- rd = r - mf (Pool), so next d2 is one DVE op ----
                   if kk + 1 < nblk * cyc:
                       rdn = wk.tile([128, FD], f32, tag="rd")
                       mf_b = r4(r[:])[:, :, :, TB - 1:TB].broadcast_to(
                           [128, BH, BL, TB]
                       )
                       nc.gpsimd.tensor_tensor(
                           out=r4(rdn[:]), in0=r4(r[:]), in1=mf_b,
                           op=Alu.subtract,
                       )

                   # ---- sumr = sum_tau r (DVE reduce) ----
                   sumr = wk.tile([128, B], f32, tag="sumr")
                   nc.vector.tensor_reduce(
                       out=sumr[:], in_=r3(r[:]),
                       axis=mybir.AxisListType.X, op=Alu.add,
                   )
                   # ---- u_raw = p^(sumr-2) (ACT exp) ----
                   u_raw = wk.tile([128, B], f32, tag="u_raw")
                   nc.scalar.activation(
                       out=u_raw[:], in_=sumr[:], func=Act.Exp,
                       bias=m2lnp_col, scale=lnp_col,
                   )

                   # ---- adaptation: a' = a*p8 + u_raw*r_end (Pool TTs) ----
                   ug = wk.tile([128, B], f32, tag="ug")
                   nc.gpsimd.tensor_tensor(
                       out=ug[:], in0=u_raw[:], in1=r_end, op=Alu.mult
                   )
                   nc.gpsimd.tensor_tensor(
                       out=a_out[:], in0=amid[:], in1=ug[:], op=Alu.add
                   )

                   # ---- v_init for next block (omf on ACT, mult on Pool) ----
                   if kk + 1 < nblk * cyc:
                       omf = wk.tile([128, B], f32, tag="omf")
                       nc.scalar.activation(
                           out=omf[:], in_=sumr[:], func=Act.Relu,
                           bias=1.0, scale=-1.0,
                       )
                       d2n = wk.tile([128, FD9], f32, tag="d2")
                       m_last = r3(m[:], TBP)[:, :, TB:TBP].squeeze(2)
                       nc.gpsimd.tensor_tensor(
                           out=r3(d2n[:], TBP)[:, :, 0:1].squeeze(2),
                           in0=omf[:], in1=m_last, op=Alu.mult,
                       )

               # ---- output spikes: transpose r, then spkT = D @ rT on PE ----
               if full:
                   rT = psout.tile([128, FD], f32, tag="rT")
                   nc.tensor.transpose(rT[:, 0:128], r[:, 0:128], ident[:])
                   nc.tensor.transpose(rT[:, 128:256], r[:, 128:256], ident[:])
                   rTs = outp.tile([128, FD], mybir.dt.bfloat16, tag="rTs")
                   nc.scalar.copy(out=rTs[:], in_=rT[:])
                   spkT = psout.tile([128, FD], f32, tag="spkT")
                   nc.tensor.matmul(spkT[:], dmat[:], rTs[:])
                   outb = outp.tile([128, FD], f32, tag="outb")
                   nc.scalar.copy(out=outb[:], in_=spkT[:])
                   nc.scalar.dma_start(
                       out=dram_block_ap(y_d[:], k), in_=outb[:]
                   )
               elif variant == "dma":
                   outb = outp.tile([128, FD], f32, tag="outb")
                   nc.vector.tensor_copy(out=outb[:], in_=xN[:])
                   nc.scalar.dma_start(
                       out=dram_block_ap(y_d[:], k), in_=outb[:]
                   )

               if do_cmp:
                   r_prev = r
                   if kk + 1 < nblk * cyc:
                       d2 = d2n
                       rd_prev = rdn

    nc.compile()
    return nc


def _host_consts(beta_raw, p_raw, b_raw, core):
    sh = slice(core * NSH, (core + 1) * NSH)
    beta = np.clip(beta_raw[sh], 0.001, 0.999).astype(np.float32)
    p = np.clip(np.abs(p_raw[sh]), 0.0, 0.999).astype(np.float32)
    bb = np.clip(np.abs(b_raw[sh]), 0.001, 1.0).astype(np.float32)

    tau = np.arange(TB, dtype=np.float32)
    bpat = np.tile(
        np.concatenate([[0.0] * 1, [1.0] * TB]).astype(np.float32)[None, :],
        (NSH, B),
    ) * beta[:, None]  # slot0 -> 0, others beta[n]
    segpat = np.tile(
        np.concatenate([[0.0], np.ones(TB - 1)]).astype(np.float32)[None, :],
        (NSH, B),
    )
    ppow = (p[:, None] ** (tau[None, :] + 1.0)).astype(np.float32)
    ppowt = np.tile((bb[:, None] * ppow).astype(np.float32), (1, B))
    lnp = np.log(p.astype(np.float64)).astype(np.float32)
    prm = np.stack(
        [(p ** 8).astype(np.float32), lnp, (-2.0 * lnp).astype(np.float32),
         np.ones_like(lnp)],
        axis=1,
    ).astype(np.float32)
    import ml_dtypes

    dmat = np.zeros((128, 128), np.float32)
    for bl in range(BL):
        for tau in range(TB):
            dmat[bl * TB + tau, bl * TB + tau] = 1.0
            if tau > 0:
                dmat[bl * TB + tau - 1, bl * TB + tau] = -1.0
    return {
        "bpat": np.ascontiguousarray(bpat),
        "segpat": np.ascontiguousarray(segpat),
        "ppowt": np.ascontiguousarray(ppowt),
        "prm": np.ascontiguousarray(prm),
        "dmat": np.ascontiguousarray(dmat.astype(ml_dtypes.bfloat16)),
    }


def build_in_maps(x, beta_raw, p_raw, b_raw, t_len=T_LEN):
    in_maps = []
    for core in range(NCORES):
        sh = slice(core * NSH, (core + 1) * NSH)
        m = {"x": np.ascontiguousarray(x[:t_len, :, sh], dtype=np.float32)}
        m.update(_host_consts(beta_raw, p_raw, b_raw, core))
        in_maps.append(m)
    return in_maps


def get_module(t_len=T_LEN, repeats=1, rep_reset=True, variant="full"):
    key = (t_len, repeats, rep_reset, variant)
    if key not in _MODULE_CACHE:
        _MODULE_CACHE[key] = _build_module(t_len, repeats, rep_reset, variant)
    return _MODULE_CACHE[key]


def kernel(x, beta_raw, p_raw, b_raw):
    _import_concourse()
    from concourse.bass_utils import run_bass_kernel_spmd

    nc = get_module(T_LEN)
    in_maps = build_in_maps(x, beta_raw, p_raw, b_raw)
    res = run_bass_kernel_spmd(nc, in_maps, core_ids=list(range(NCORES)))
    y = np.concatenate([res.results[c]["y"] for c in range(NCORES)], axis=2)
    return y.astype(np.float32)


if __name__ == "__main__":
    xs = np.random.RandomState(0).randn(T_LEN, B, N).astype(np.float32) * 0.6
    br = np.random.RandomState(1).uniform(0.7, 0.99, N).astype(np.float32)
    pr = np.random.RandomState(2).uniform(0.5, 0.95, N).astype(np.float32)
    brw = np.random.RandomState(3).uniform(0.2, 1.0, N).astype(np.float32)
    out = kernel(xs, br, pr, brw)
    print(out.shape, out.dtype, out.mean())
